# revision 15
# baseline (speedup 1.0000x reference)
"""AdditiveAttention (Bahdanau) Trainium2 kernel — 8-core data-parallel.

Math: scores[b,q,k] = sum_h wv[h] * tanh(qf[b,q,h] + kf[b,k,h]),
      out = softmax_k(mask(scores)) @ values.

tanh(a+b) is a density-weighted least-squares Fourier sine series
tanh(x) ~= sum_m b_m sin(2*pi*m*x/(2L)), which separates via
sin(m(A+B)) = sin(mA)cos(mB) + cos(mA)sin(mB).  Per term m the kernel
needs one weighted-sin tensor and one cos tensor per side; the bilinear
form sg*c is invariant under (sg/l, l*c), so all b_m coefficient ratios
fold into single-instruction custom DVE polynomials evaluated straight
from s1 = sin(2*pi*p) and h = sin(pi*p) (both in ACT Sin's domain by
choice of half-period; no range reduction anywhere):
    c1   = 1-2h^2                 COSQ(h)
    c2'  = a2(1-2s1^2)            COSQ(s1)
    c4'  = g4(2c2^2-1)            COSQ(c2')
    sg3' = sg1(3-4s1^2)           P1(sg1, s1)     [= b1*wv*s3]
    c3'  = (b3/b1)c1(1-4s1^2)     P1(c1, s1)
    sg2' = sg1*c1 (TT)            sg4' = sg2'*c2' (TT)
    m=5 via sin5 = s1*(16u^2-20u+5), cos5 = c1*(16u^2-12u+1), u = s1^2:
    the quadratics are ACT Square of an affine, (4u-c)^2 - 1.25, then an
    ACT affine Copy, then a plain TT against sg1/c1 on GPSIMD.
All elementwise work is statically scheduled across DVE/ACT/GPSIMD to
minimize total engine-activity (the HAM power duty-cycle throttles all
engines when aggregate activity is high).  fp16 throughout; softmax
needs no max pass; masking is an additive -1e6 exp bias; the softmax
denominator is a ones-column in the values matmul.
"""
import sys

sys.path.insert(0, "/opt/trn_rl_repo")

import numpy as np

from concourse import bacc, bass, dve_ops, mybir, tile
from concourse.bass_utils import run_bass_kernel_spmd
from concourse.tile_rust import add_dep_helper
from concourse.dve_spec import Spec, Src0, Src1, C0, C1, C2, lower
from concourse.dve_spec import _has_src1 as has_src1
from concourse.dve_uop import DveOpSpec

N_CORES = 8
B, Q, K, D, H = 16, 256, 256, 256, 256
SLOTS = B // N_CORES  # 2 batches per core
M_TERMS = 5
L_OVER_XM = 1.10  # half-period / data range
MASK_NEG = -1.0e6
PI = float(np.pi)
TWO_PI = float(2 * np.pi)
N_WARM = 6

LAST_EXEC_TIME_NS = None
LAST_RESULTS = None

F32 = mybir.dt.float32
F16 = mybir.dt.float16
BF16 = mybir.dt.bfloat16
AF = mybir.ActivationFunctionType
MULT = mybir.AluOpType.mult
ADD = mybir.AluOpType.add


# -------------------------------------------------------- custom DVE ops
def _cosq_ref(in0, in1, s0, s1, imm2):
    x = in0.astype(np.float32)
    return (np.float32(s0) + np.float32(s1) * x * x).astype(np.float32)


def _p1_ref(in0, in1, s0, s1, imm2):
    a = in0.astype(np.float32)
    x = in1.astype(np.float32)
    return (a * (np.float32(s0) + np.float32(s1) * x * x)).astype(np.float32)


def _p2_ref(in0, in1, s0, s1, imm2):
    a = in0.astype(np.float32)
    u = in1.astype(np.float32) ** 2
    return (
        a * (np.float32(s0) + u * (np.float32(s1) + np.float32(imm2) * u))
    ).astype(np.float32)


_OP_BODIES = {
    "COSQ_ANT": (lambda: C0 + (Src0 * Src0) * C1, _cosq_ref),
    "POLY1_ANT": (lambda: Src0 * (C0 + (Src1 * Src1) * C1), _p1_ref),
    "POLY2_ANT": (
        lambda: Src0 * (C0 + (Src1 * Src1) * (C1 + C2 * (Src1 * Src1))),
        _p2_ref,
    ),
}


def _register_ops():
    ops = {}
    for name, (body_fn, ref) in _OP_BODIES.items():
        if name in dve_ops._SUB_OPCODE_FOR_NAME:
            for op in dve_ops.OPS:
                if op.name == name:
                    ops[name] = op
                    break
            continue
        spec = Spec(body=body_fn(), reference=ref)
        opcode = 1 + len(dve_ops.OPS)
        assert opcode < 0x20
        dve_ops._SUB_OPCODE_FOR_NAME[name] = opcode
        shas = {
            ver: DveOpSpec(
                name=name, opcode=opcode, uops=lower(spec, ver=ver),
                rd1_en=has_src1(spec),
            ).sha(ver)
            for ver in ("v3", "v4")
        }
        op = dve_ops.DveOp(name, spec, subdim=False, uops_sha=shas)
        dve_ops.OPS.append(op)
        dve_ops.CUSTOM_DVE_SPECS[name] = spec
        ops[name] = op
    return ops


# ------------------------------------------------------------- Fourier fit
def _fit_coeffs(xm, m_terms, half_period, sig):
    x = np.linspace(-xm, xm, 6001)
    w0 = np.pi / half_period
    A = np.stack([np.sin(m * w0 * x) for m in range(1, m_terms + 1)], axis=1)
    sig_mult, floor = (1.1, 0.003) if m_terms == 4 else (1.0, 0.01)
    wgt = np.sqrt(np.exp(-0.5 * (x / (sig * sig_mult)) ** 2) + floor)
    coef, *_ = np.linalg.lstsq(A * wgt[:, None], np.tanh(x) * wgt, rcond=None)
    return coef.astype(np.float64)


# ------------------------------------------------------------- graph build
def _build_graph(coef, ops):
    bm = [float(c) for c in coef]
    b1, b2, b3, b4 = bm[0], bm[1], bm[2], bm[3]
    b5 = bm[4] if M_TERMS >= 5 else 0.0
    COSQ, P1 = ops["COSQ_ANT"], ops["POLY1_ANT"]
    nc = bacc.Bacc("TRN2", target_bir_lowering=False, debug=False)

    qkT = nc.dram_tensor("qkT", [SLOTS, 128, 2, 2, Q], F16, kind="ExternalInput")
    wqk = nc.dram_tensor("wqk", [128, 2, 2, H], F16, kind="ExternalInput")
    vals = nc.dram_tensor("vals", [SLOTS, 128, 2, D + 1], F16, kind="ExternalInput")
    aux = nc.dram_tensor("aux", [128, 8], F32, kind="ExternalInput")
    out = nc.dram_tensor("out", [SLOTS, Q, D], F32, kind="ExternalOutput")

    a2 = 2.0 * b2 / b1  # c2' = a2 * c2
    g4 = 2.0 * b4 / b2  # c4' = g4 * c4
    be5 = b5 / b1

    with tile.TileContext(nc) as tc:
        with (
            tc.tile_pool(name="w", bufs=1) as wpool,
            tc.tile_pool(name="io", bufs=2) as iopool,
            tc.tile_pool(name="trig", bufs=2) as trig,
            tc.tile_pool(name="fin", bufs=2) as fin,
            tc.tile_pool(name="psp", bufs=2, space="PSUM") as ps_pall,
            tc.tile_pool(name="pss", bufs=2, space="PSUM") as ps_scores,
            tc.tile_pool(name="pso", bufs=2, space="PSUM") as ps_out,
        ):
            # ---- PE warmup during the DMA window (DVFS clock ramp).
            scratch = wpool.tile([128, 512], F16, tag="scratch")
            nc.vector.memset(scratch[:], 0.0)
            warm_ps = ps_out.tile([64, 512], F32, tag="out")
            for _wi in range(N_WARM):
                nc.tensor.matmul(
                    warm_ps[:], scratch[:, 0:64], scratch[:],
                    start=(_wi == 0), stop=(_wi == N_WARM - 1),
                    skip_group_check=True,
                )

            # ---- input DMAs: one large descriptor per tensor per slot.
            wqk_sb = wpool.tile([128, 2, 2, H], F16, tag="wqk")
            nc.sync.dma_start(wqk_sb[:], wqk[:])
            qk_sbs = []
            for b in range(SLOTS):
                qk_t = iopool.tile([128, 2, 2, Q], F16, tag="qk")
                nc.sync.dma_start(qk_t[:], qkT[b])
                qk_sbs.append(qk_t)
            aux_sb = wpool.tile([128, 8], F32, tag="aux")
            nc.sync.dma_start(aux_sb[:], aux[:])
            vals_sbs = []
            for b in range(SLOTS):
                vals_sb = iopool.tile([128, 2, D + 1], F16, tag="vals")
                nc.sync.dma_start(vals_sb[:], vals[b])
                vals_sbs.append(vals_sb)

            # ---- projections into PSUM; blk = hc*2 + side.
            all_p_all = []
            for b in range(SLOTS):
                p_all = ps_pall.tile([128, 4, 256], F32, tag="pall")
                all_p_all.append(p_all)

            def emit_proj(b):
                for j in range(2):
                    for side in range(2):
                        for hc in range(2):
                            blk = hc * 2 + side
                            nc.tensor.matmul(
                                all_p_all[b][:, blk, :],
                                wqk_sb[:, side, j, hc * 128 : (hc + 1) * 128],
                                qk_sbs[b][:, side, j, :],
                                start=(j == 0 and side == 0),
                                stop=(j == 1 and side == 1),
                                skip_group_check=True,
                            )

            emit_proj(0)
            emit_proj(1)

            def T(tag):
                return [trig.tile([128, 4, 256], F16, tag=tag, name=f"{tag}{b}")
                        for b in range(SLOTS)]

            h_t, s1_t, g1_t = T("h"), T("s1"), T("g1")
            c_t = {m: T(f"c{m}") for m in range(1, M_TERMS + 1)}
            g_t = {m: T(f"g{m}") for m in range(2, M_TERMS + 1)}
            u2_t, x_t, y_t = T("u2"), T("x"), T("y")
            if M_TERMS >= 5:
                w5_t, w5c_t, v5_t, v5c_t = T("w5"), T("w5c"), T("v5"), T("v5c")

            def flat(t):
                return t[:].rearrange("p a b -> p (a b)")

            # ACT sines (the only table function until the exps); sg1
            # weighting immediately after each slot's sines so the m=1
            # matmuls can start while the other slot's sines run.
            act_chain = []
            for b in range(SLOTS):
                p_flat = all_p_all[b][:].rearrange("p a b -> p (a b)")
                act_chain.append(nc.scalar.activation(
                    flat(h_t[b]), p_flat, AF.Sin, scale=PI))
                act_chain.append(nc.scalar.activation(
                    flat(s1_t[b]), p_flat, AF.Sin, scale=TWO_PI))
                for hc in range(2):
                    act_chain.append(nc.scalar.mul(
                        g1_t[b][:, 2 * hc : 2 * hc + 2, :],
                        s1_t[b][:, 2 * hc : 2 * hc + 2, :],
                        aux_sb[:, hc : hc + 1],
                    ))
            for _p, _n in zip(act_chain, act_chain[1:]):
                add_dep_helper(_n.ins, _p.ins, sync=False,
                               reason="pin ACT issue order")

            # elementwise chain
            V, G, A = nc.vector, nc.gpsimd, nc.scalar
            for b in range(SLOTS):
                G.tensor_mul(flat(u2_t[b]), flat(s1_t[b]), flat(s1_t[b]))
                V._custom_dve(COSQ, out=flat(c_t[1][b]), in0=flat(h_t[b]),
                              s0=1.0, s1=-2.0)
                V._custom_dve(COSQ, out=flat(c_t[2][b]), in0=flat(s1_t[b]),
                              s0=a2, s1=-2.0 * a2)
                V.tensor_mul(flat(g_t[2][b]), flat(g1_t[b]), flat(c_t[1][b]))
                G.tensor_scalar(flat(x_t[b]), flat(u2_t[b]), -4.0, 3.0,
                                MULT, ADD)
                G.tensor_scalar(flat(y_t[b]), flat(u2_t[b]),
                                -4.0 * b3 / b1, b3 / b1, MULT, ADD)
                V.tensor_mul(flat(g_t[3][b]), flat(g1_t[b]), flat(x_t[b]))
                V.tensor_mul(flat(c_t[3][b]), flat(c_t[1][b]), flat(y_t[b]))
                V._custom_dve(COSQ, out=flat(c_t[4][b]), in0=flat(c_t[2][b]),
                              s0=-g4, s1=2.0 * g4 / (a2 * a2))
                V.tensor_mul(flat(g_t[4][b]), flat(g_t[2][b]), flat(c_t[2][b]))
                if M_TERMS >= 5:
                    A.activation(flat(w5_t[b]), flat(u2_t[b]), AF.Square,
                                 bias=aux_sb[:, 6:7], scale=4.0)
                    A.activation(flat(w5c_t[b]), flat(u2_t[b]), AF.Square,
                                 bias=aux_sb[:, 7:8], scale=4.0)
                    A.activation(flat(v5_t[b]), flat(w5_t[b]), AF.Copy,
                                 bias=-1.25, scale=1.0)
                    A.activation(flat(v5c_t[b]), flat(w5c_t[b]), AF.Copy,
                                 bias=-1.25 * be5, scale=be5)
                    V.tensor_mul(flat(g_t[5][b]), flat(g1_t[b]), flat(v5_t[b]))
                    V.tensor_mul(flat(c_t[5][b]), flat(c_t[1][b]), flat(v5c_t[b]))

            # ---- score matmuls accumulate all terms into ps_sT per slot.
            all_ps_sT = []
            for b in range(SLOTS):
                ps_sT = ps_scores.tile([128, 2, Q], F32, tag="scores")
                all_ps_sT.append(ps_sT)

            sg_tiles = {1: g1_t, **{m: g_t[m] for m in range(2, M_TERMS + 1)}}

            mm_order = [(m, b) for b in range(SLOTS)
                        for m in range(1, M_TERMS + 1)]
            for mi, b in mm_order:
                sg, cm = sg_tiles[mi][b], c_t[mi][b]
                ps_sT = all_ps_sT[b]
                first = mi == 1
                last = mi == M_TERMS
                for kc in range(2):
                    ksl = slice(kc * 128, kc * 128 + 128)
                    for hc in range(2):
                        nc.tensor.matmul(
                            ps_sT[:, kc, :], cm[:, 2 * hc + 1, ksl],
                            sg[:, 2 * hc, :],
                            start=(first and kc == 0 and hc == 0),
                            stop=False,
                            skip_group_check=True,
                        )
                        nc.tensor.matmul(
                            ps_sT[:, kc, :], sg[:, 2 * hc + 1, ksl],
                            cm[:, 2 * hc, :],
                            start=False,
                            stop=(last and kc == 1 and hc == 1),
                            skip_group_check=True,
                        )

            # ---- masked exp, output matmuls, normalization, DMA out.
            expT = {}
            for b in range(SLOTS):
                for kc in range(2):
                    e = fin.tile([128, Q], F16, tag="expT", name=f"expT{b}{kc}",
                                 bufs=4)
                    nc.scalar.activation(
                        e[:], all_ps_sT[b][:, kc, :], AF.Exp,
                        bias=aux_sb[:, 2 + 2 * b + kc : 3 + 2 * b + kc],
                    )
                    expT[(b, kc)] = e

            for b in range(SLOTS):
                for qt in range(2):
                    po = ps_out.tile([128, D + 1], F32, tag="out", name=f"po{b}{qt}")
                    for kc in range(2):
                        nc.tensor.matmul(
                            po[:],
                            expT[(b, kc)][:, qt * 128 : (qt + 1) * 128],
                            vals_sbs[b][:, kc, :],
                            start=(kc == 0),
                            stop=(kc == 1),
                        )
                    recip = fin.tile([128, 1], F32, tag="recip", name=f"rc{b}{qt}",
                                     bufs=4)
                    nc.vector.reciprocal(recip[:], po[:, D : D + 1])
                    out_sb = fin.tile([128, D], F32, tag="outsb",
                                      name=f"osb{b}{qt}", bufs=4)
                    nc.vector.tensor_scalar_mul(out_sb[:], po[:, 0:D], recip[:])
                    nc.sync.dma_start(
                        out[b, qt * 128 : (qt + 1) * 128, :], out_sb[:]
                    )

    nc.compile()
    return nc


_CACHED = {}


def _get_graph(coef):
    key = tuple(np.round(coef, 12))
    if key not in _CACHED:
        ops = _register_ops()
        _CACHED[key] = _build_graph(coef, ops)
    return _CACHED[key]


def _prepare(inputs):
    queries = np.ascontiguousarray(np.asarray(inputs["queries"], dtype=np.float32))
    keys = np.ascontiguousarray(np.asarray(inputs["keys"], dtype=np.float32))
    values = np.ascontiguousarray(np.asarray(inputs["values"], dtype=np.float32))
    valid_lens = np.asarray(inputs["valid_lens"]).astype(np.int64)
    Wq = np.asarray(inputs["Wq"], dtype=np.float32)
    Wk = np.asarray(inputs["Wk"], dtype=np.float32)
    wv = np.asarray(inputs["wv"], dtype=np.float32)

    qf = queries.reshape(-1, D) @ Wq
    kf = keys.reshape(-1, D) @ Wk
    xm = (float(np.abs(qf).max()) + float(np.abs(kf).max())) * 1.02
    sig = float(np.sqrt(qf.std() ** 2 + kf.std() ** 2))
    half_period = L_OVER_XM * xm
    coef = _fit_coeffs(xm, M_TERMS, half_period, sig)
    scale = 1.0 / (2.0 * half_period)

    qT = queries.transpose(0, 2, 1).reshape(B, 2, 128, Q).transpose(0, 2, 1, 3)
    kT = keys.transpose(0, 2, 1).reshape(B, 2, 128, K).transpose(0, 2, 1, 3)
    qkT_np = np.ascontiguousarray(np.stack([qT, kT], axis=2).astype(np.float16))
    wq = (Wq * scale).reshape(2, 128, H).transpose(1, 0, 2)
    wk = (Wk * scale).reshape(2, 128, H).transpose(1, 0, 2)
    wqk_np = np.ascontiguousarray(np.stack([wq, wk], axis=1).astype(np.float16))
    ones = np.ones((B, K, 1), np.float32)
    vals_np = np.ascontiguousarray(
        np.concatenate([values, ones], axis=2)
        .reshape(B, 2, 128, D + 1)
        .transpose(0, 2, 1, 3)
        .astype(np.float16)
    )
    beta1 = (float(coef[0]) * wv).reshape(2, 128)
    kidx = np.arange(K)
    maskv = np.where(
        kidx[None, :] < valid_lens[:, None], 0.0, MASK_NEG
    ).astype(np.float32).reshape(B, 2, 128)
    aux_np = np.zeros((N_CORES, 128, 8), np.float32)
    aux_np[:, :, 6] = -2.5
    aux_np[:, :, 7] = -1.5
    for c in range(N_CORES):
        aux_np[c, :, 0] = beta1[0]
        aux_np[c, :, 1] = beta1[1]
        for sl in range(SLOTS):
            for kc in range(2):
                aux_np[c, :, 2 + 2 * sl + kc] = maskv[c * SLOTS + sl, kc]

    return {
        "qkT": qkT_np,
        "wqk": wqk_np,
        "vals": vals_np,
        "aux": aux_np,
        "coef": coef,
    }


def kernel(**inputs) -> np.ndarray:
    global LAST_EXEC_TIME_NS, LAST_RESULTS
    g = _prepare(inputs)
    nc = _get_graph(g["coef"])
    in_maps = []
    for c in range(N_CORES):
        sl = slice(c * SLOTS, (c + 1) * SLOTS)
        in_maps.append(
            {
                "qkT": g["qkT"][sl],
                "wqk": g["wqk"],
                "vals": g["vals"][sl],
                "aux": g["aux"][c],
            }
        )

    res = run_bass_kernel_spmd(nc, in_maps, core_ids=list(range(N_CORES)))
    LAST_EXEC_TIME_NS = res.exec_time_ns
    LAST_RESULTS = res
    out = np.concatenate(
        [np.asarray(res.results[c]["out"]) for c in range(N_CORES)], axis=0
    )
    return out.astype(np.float32)


if __name__ == "__main__":
    import os

    if os.path.exists("/root/problem/inputs_cache.npz"):
        d = np.load("/root/problem/inputs_cache.npz")
        o = kernel(**{k: d[k] for k in d.files})
        exp = np.load("/root/problem/expected_cache.npy")
        rel = np.linalg.norm(o - exp) / np.linalg.norm(exp)
        relmax = np.abs(o - exp).max() / np.abs(exp).max()
        print("rel norm err:", rel, "rel max err:", relmax)


# revision 16
# speedup vs baseline: 1.1811x; 1.1811x over previous
"""AdditiveAttention (Bahdanau) Trainium2 kernel — 8-core data-parallel.

Math: scores[b,q,k] = sum_h wv[h] * tanh(qf[b,q,h] + kf[b,k,h]),
      out = softmax_k(mask(scores)) @ values.

tanh(a+b) is a density-weighted least-squares Fourier sine series
tanh(x) ~= sum_m b_m sin(2*pi*m*x/(2L)), which separates via
sin(m(A+B)) = sin(mA)cos(mB) + cos(mA)sin(mB).  Per term m the kernel
needs one weighted-sin tensor and one cos tensor per side; the bilinear
form sg*c is invariant under (sg/l, l*c), so all b_m coefficient ratios
fold into single-instruction custom DVE polynomials evaluated straight
from s1 = sin(2*pi*p) and h = sin(pi*p) (both in ACT Sin's domain by
choice of half-period; no range reduction anywhere):
    c1   = 1-2h^2                 COSQ(h)
    c2'  = a2(1-2s1^2)            COSQ(s1)
    c4'  = g4(2c2^2-1)            COSQ(c2')
    sg3' = sg1(3-4s1^2)           P1(sg1, s1)     [= b1*wv*s3]
    c3'  = (b3/b1)c1(1-4s1^2)     P1(c1, s1)
    sg2' = sg1*c1 (TT)            sg4' = sg2'*c2' (TT)
    m=5 via sin5 = s1*(16u^2-20u+5), cos5 = c1*(16u^2-12u+1), u = s1^2:
    the quadratics are ACT Square of an affine, (4u-c)^2 - 1.25, then an
    ACT affine Copy, then a plain TT against sg1/c1 on GPSIMD.
All elementwise work is statically scheduled across DVE/ACT/GPSIMD to
minimize total engine-activity (the HAM power duty-cycle throttles all
engines when aggregate activity is high).  fp16 throughout; softmax
needs no max pass; masking is an additive -1e6 exp bias; the softmax
denominator is a ones-column in the values matmul.
"""
import sys

sys.path.insert(0, "/opt/trn_rl_repo")

import numpy as np

from concourse import bacc, bass, dve_ops, mybir, tile
from concourse.bass_utils import run_bass_kernel_spmd
from concourse.tile_rust import add_dep_helper
from concourse.dve_spec import Spec, Src0, Src1, C0, C1, C2, lower
from concourse.dve_spec import _has_src1 as has_src1
from concourse.dve_uop import DveOpSpec

N_CORES = 8
B, Q, K, D, H = 16, 256, 256, 256, 256
SLOTS = B // N_CORES  # 2 batches per core
M_TERMS = 5
L_OVER_XM = 1.10  # half-period / data range
MASK_NEG = -1.0e6
PI = float(np.pi)
TWO_PI = float(2 * np.pi)
N_WARM = 6

LAST_EXEC_TIME_NS = None
LAST_RESULTS = None

F32 = mybir.dt.float32
F16 = mybir.dt.float16
BF16 = mybir.dt.bfloat16
AF = mybir.ActivationFunctionType
MULT = mybir.AluOpType.mult
ADD = mybir.AluOpType.add


# -------------------------------------------------------- custom DVE ops
def _cosq_ref(in0, in1, s0, s1, imm2):
    x = in0.astype(np.float32)
    return (np.float32(s0) + np.float32(s1) * x * x).astype(np.float32)


def _p1_ref(in0, in1, s0, s1, imm2):
    a = in0.astype(np.float32)
    x = in1.astype(np.float32)
    return (a * (np.float32(s0) + np.float32(s1) * x * x)).astype(np.float32)


def _p2_ref(in0, in1, s0, s1, imm2):
    a = in0.astype(np.float32)
    u = in1.astype(np.float32) ** 2
    return (
        a * (np.float32(s0) + u * (np.float32(s1) + np.float32(imm2) * u))
    ).astype(np.float32)


_OP_BODIES = {
    "COSQ_ANT": (lambda: C0 + (Src0 * Src0) * C1, _cosq_ref),
    "POLY1_ANT": (lambda: Src0 * (C0 + (Src1 * Src1) * C1), _p1_ref),
    "POLY2_ANT": (
        lambda: Src0 * (C0 + (Src1 * Src1) * (C1 + C2 * (Src1 * Src1))),
        _p2_ref,
    ),
}


def _register_ops():
    ops = {}
    for name, (body_fn, ref) in _OP_BODIES.items():
        if name in dve_ops._SUB_OPCODE_FOR_NAME:
            for op in dve_ops.OPS:
                if op.name == name:
                    ops[name] = op
                    break
            continue
        spec = Spec(body=body_fn(), reference=ref)
        opcode = 1 + len(dve_ops.OPS)
        assert opcode < 0x20
        dve_ops._SUB_OPCODE_FOR_NAME[name] = opcode
        shas = {
            ver: DveOpSpec(
                name=name, opcode=opcode, uops=lower(spec, ver=ver),
                rd1_en=has_src1(spec),
            ).sha(ver)
            for ver in ("v3", "v4")
        }
        op = dve_ops.DveOp(name, spec, subdim=False, uops_sha=shas)
        dve_ops.OPS.append(op)
        dve_ops.CUSTOM_DVE_SPECS[name] = spec
        ops[name] = op
    return ops


# ------------------------------------------------------------- Fourier fit
def _fit_coeffs(xm, m_terms, half_period, sig):
    x = np.linspace(-xm, xm, 6001)
    w0 = np.pi / half_period
    A = np.stack([np.sin(m * w0 * x) for m in range(1, m_terms + 1)], axis=1)
    sig_mult, floor = (1.1, 0.003) if m_terms == 4 else (1.0, 0.01)
    wgt = np.sqrt(np.exp(-0.5 * (x / (sig * sig_mult)) ** 2) + floor)
    coef, *_ = np.linalg.lstsq(A * wgt[:, None], np.tanh(x) * wgt, rcond=None)
    return coef.astype(np.float64)


# ------------------------------------------------------------- graph build
def _build_graph(coef, ops):
    bm = [float(c) for c in coef]
    b1, b2, b3, b4 = bm[0], bm[1], bm[2], bm[3]
    b5 = bm[4] if M_TERMS >= 5 else 0.0
    COSQ, P1 = ops["COSQ_ANT"], ops["POLY1_ANT"]
    nc = bacc.Bacc("TRN2", target_bir_lowering=False, debug=False)

    qkT = nc.dram_tensor("qkT", [SLOTS, 128, 2, 2, Q], F16, kind="ExternalInput")
    wqk = nc.dram_tensor("wqk", [128, 2, 2, H], F16, kind="ExternalInput")
    vals = nc.dram_tensor("vals", [SLOTS, 128, 2, D + 1], F16, kind="ExternalInput")
    aux = nc.dram_tensor("aux", [128, 8], F32, kind="ExternalInput")
    out = nc.dram_tensor("out", [SLOTS, Q, D], F32, kind="ExternalOutput")

    a2 = 2.0 * b2 / b1  # c2' = a2 * c2
    g4 = 2.0 * b4 / b2  # c4' = g4 * c4
    be5 = b5 / b1

    with tile.TileContext(nc) as tc:
        with (
            tc.tile_pool(name="w", bufs=1) as wpool,
            tc.tile_pool(name="io", bufs=2) as iopool,
            tc.tile_pool(name="trig", bufs=2) as trig,
            tc.tile_pool(name="fin", bufs=2) as fin,
            tc.tile_pool(name="psp", bufs=2, space="PSUM") as ps_pall,
            tc.tile_pool(name="pss", bufs=2, space="PSUM") as ps_scores,
            tc.tile_pool(name="pso", bufs=2, space="PSUM") as ps_out,
        ):
            # ---- PE warmup during the DMA window (DVFS clock ramp).
            scratch = wpool.tile([128, 512], F16, tag="scratch")
            nc.vector.memset(scratch[:], 0.0)
            warm_ps = ps_out.tile([64, 512], F32, tag="out")
            for _wi in range(N_WARM):
                nc.tensor.matmul(
                    warm_ps[:], scratch[:, 0:64], scratch[:],
                    start=(_wi == 0), stop=(_wi == N_WARM - 1),
                    skip_group_check=True,
                )

            # ---- input DMAs: one large descriptor per tensor per slot.
            wqk_sb = wpool.tile([128, 2, 2, H], F16, tag="wqk")
            nc.sync.dma_start(wqk_sb[:], wqk[:])
            qk_sbs = []
            for b in range(SLOTS):
                qk_t = iopool.tile([128, 2, 2, Q], F16, tag="qk")
                nc.sync.dma_start(qk_t[:], qkT[b])
                qk_sbs.append(qk_t)
            aux_sb = wpool.tile([128, 8], F32, tag="aux")
            nc.sync.dma_start(aux_sb[:], aux[:])
            vals_sbs = []
            for b in range(SLOTS):
                vals_sb = iopool.tile([128, 2, D + 1], F16, tag="vals")
                nc.sync.dma_start(vals_sb[:], vals[b])
                vals_sbs.append(vals_sb)

            # ---- projections into PSUM; blk = hc*2 + side.
            all_p_all = []
            for b in range(SLOTS):
                p_all = ps_pall.tile([128, 4, 256], F32, tag="pall")
                all_p_all.append(p_all)

            def emit_proj(b):
                for j in range(2):
                    for side in range(2):
                        for hc in range(2):
                            blk = hc * 2 + side
                            nc.tensor.matmul(
                                all_p_all[b][:, blk, :],
                                wqk_sb[:, side, j, hc * 128 : (hc + 1) * 128],
                                qk_sbs[b][:, side, j, :],
                                start=(j == 0 and side == 0),
                                stop=(j == 1 and side == 1),
                                skip_group_check=True,
                            )

            emit_proj(0)
            emit_proj(1)

            def T(tag):
                return [trig.tile([128, 4, 256], BF16, tag=tag, name=f"{tag}{b}")
                        for b in range(SLOTS)]

            h_t, s1_t, g1_t = T("h"), T("s1"), T("g1")
            c_t = {m: T(f"c{m}") for m in range(1, M_TERMS + 1)}
            g_t = {m: T(f"g{m}") for m in range(2, M_TERMS + 1)}
            u2_t, x_t, y_t = T("u2"), T("x"), T("y")
            if M_TERMS >= 5:
                w5_t, w5c_t, v5_t, v5c_t = T("w5"), T("w5c"), T("v5"), T("v5c")

            def flat(t):
                return t[:].rearrange("p a b -> p (a b)")

            # ACT sines (the only table function until the exps); sg1
            # weighting immediately after each slot's sines so the m=1
            # matmuls can start while the other slot's sines run.
            act_chain = []
            for b in range(SLOTS):
                p_flat = all_p_all[b][:].rearrange("p a b -> p (a b)")
                act_chain.append(nc.scalar.activation(
                    flat(h_t[b]), p_flat, AF.Sin, scale=PI))
                act_chain.append(nc.scalar.activation(
                    flat(s1_t[b]), p_flat, AF.Sin, scale=TWO_PI))
                for hc in range(2):
                    act_chain.append(nc.scalar.mul(
                        g1_t[b][:, 2 * hc : 2 * hc + 2, :],
                        s1_t[b][:, 2 * hc : 2 * hc + 2, :],
                        aux_sb[:, hc : hc + 1],
                    ))
            for _p, _n in zip(act_chain, act_chain[1:]):
                add_dep_helper(_n.ins, _p.ins, sync=False,
                               reason="pin ACT issue order")

            # elementwise chain
            V, G, A = nc.vector, nc.gpsimd, nc.scalar
            for b in range(SLOTS):
                G.tensor_mul(flat(u2_t[b]), flat(s1_t[b]), flat(s1_t[b]))
                V._custom_dve(COSQ, out=flat(c_t[1][b]), in0=flat(h_t[b]),
                              s0=1.0, s1=-2.0)
                V._custom_dve(COSQ, out=flat(c_t[2][b]), in0=flat(s1_t[b]),
                              s0=a2, s1=-2.0 * a2)
                V.tensor_mul(flat(g_t[2][b]), flat(g1_t[b]), flat(c_t[1][b]))
                G.tensor_scalar(flat(x_t[b]), flat(u2_t[b]), -4.0, 3.0,
                                MULT, ADD)
                G.tensor_scalar(flat(y_t[b]), flat(u2_t[b]),
                                -4.0 * b3 / b1, b3 / b1, MULT, ADD)
                V.tensor_mul(flat(g_t[3][b]), flat(g1_t[b]), flat(x_t[b]))
                V.tensor_mul(flat(c_t[3][b]), flat(c_t[1][b]), flat(y_t[b]))
                V._custom_dve(COSQ, out=flat(c_t[4][b]), in0=flat(c_t[2][b]),
                              s0=-g4, s1=2.0 * g4 / (a2 * a2))
                V.tensor_mul(flat(g_t[4][b]), flat(g_t[2][b]), flat(c_t[2][b]))
                if M_TERMS >= 5:
                    A.activation(flat(w5_t[b]), flat(u2_t[b]), AF.Square,
                                 bias=aux_sb[:, 6:7], scale=4.0)
                    A.activation(flat(w5c_t[b]), flat(u2_t[b]), AF.Square,
                                 bias=aux_sb[:, 7:8], scale=4.0)
                    A.activation(flat(v5_t[b]), flat(w5_t[b]), AF.Copy,
                                 bias=-1.25, scale=1.0)
                    A.activation(flat(v5c_t[b]), flat(w5c_t[b]), AF.Copy,
                                 bias=-1.25 * be5, scale=be5)
                    V.tensor_mul(flat(g_t[5][b]), flat(g1_t[b]), flat(v5_t[b]))
                    V.tensor_mul(flat(c_t[5][b]), flat(c_t[1][b]), flat(v5c_t[b]))

            # ---- score matmuls accumulate all terms into ps_sT per slot.
            all_ps_sT = []
            for b in range(SLOTS):
                ps_sT = ps_scores.tile([128, 2, Q], F32, tag="scores")
                all_ps_sT.append(ps_sT)

            sg_tiles = {1: g1_t, **{m: g_t[m] for m in range(2, M_TERMS + 1)}}

            mm_order = [(m, b) for b in range(SLOTS)
                        for m in range(1, M_TERMS + 1)]
            for mi, b in mm_order:
                sg, cm = sg_tiles[mi][b], c_t[mi][b]
                ps_sT = all_ps_sT[b]
                first = mi == 1
                last = mi == M_TERMS
                for kc in range(2):
                    ksl = slice(kc * 128, kc * 128 + 128)
                    for hc in range(2):
                        nc.tensor.matmul(
                            ps_sT[:, kc, :], cm[:, 2 * hc + 1, ksl],
                            sg[:, 2 * hc, :],
                            start=(first and kc == 0 and hc == 0),
                            stop=False,
                            skip_group_check=True,
                        )
                        nc.tensor.matmul(
                            ps_sT[:, kc, :], sg[:, 2 * hc + 1, ksl],
                            cm[:, 2 * hc, :],
                            start=False,
                            stop=(last and kc == 1 and hc == 1),
                            skip_group_check=True,
                        )

            # ---- masked exp, output matmuls, normalization, DMA out.
            expT = {}
            for b in range(SLOTS):
                for kc in range(2):
                    e = fin.tile([128, Q], F16, tag="expT", name=f"expT{b}{kc}",
                                 bufs=4)
                    nc.scalar.activation(
                        e[:], all_ps_sT[b][:, kc, :], AF.Exp,
                        bias=aux_sb[:, 2 + 2 * b + kc : 3 + 2 * b + kc],
                    )
                    expT[(b, kc)] = e

            for b in range(SLOTS):
                for qt in range(2):
                    po = ps_out.tile([128, D + 1], F32, tag="out", name=f"po{b}{qt}")
                    for kc in range(2):
                        nc.tensor.matmul(
                            po[:],
                            expT[(b, kc)][:, qt * 128 : (qt + 1) * 128],
                            vals_sbs[b][:, kc, :],
                            start=(kc == 0),
                            stop=(kc == 1),
                        )
                    recip = fin.tile([128, 1], F32, tag="recip", name=f"rc{b}{qt}",
                                     bufs=4)
                    nc.vector.reciprocal(recip[:], po[:, D : D + 1])
                    out_sb = fin.tile([128, D], F32, tag="outsb",
                                      name=f"osb{b}{qt}", bufs=4)
                    nc.vector.tensor_scalar_mul(out_sb[:], po[:, 0:D], recip[:])
                    nc.sync.dma_start(
                        out[b, qt * 128 : (qt + 1) * 128, :], out_sb[:]
                    )

    nc.compile()
    return nc


_CACHED = {}


def _get_graph(coef):
    key = tuple(np.round(coef, 12))
    if key not in _CACHED:
        ops = _register_ops()
        _CACHED[key] = _build_graph(coef, ops)
    return _CACHED[key]


def _prepare(inputs):
    queries = np.ascontiguousarray(np.asarray(inputs["queries"], dtype=np.float32))
    keys = np.ascontiguousarray(np.asarray(inputs["keys"], dtype=np.float32))
    values = np.ascontiguousarray(np.asarray(inputs["values"], dtype=np.float32))
    valid_lens = np.asarray(inputs["valid_lens"]).astype(np.int64)
    Wq = np.asarray(inputs["Wq"], dtype=np.float32)
    Wk = np.asarray(inputs["Wk"], dtype=np.float32)
    wv = np.asarray(inputs["wv"], dtype=np.float32)

    qf = queries.reshape(-1, D) @ Wq
    kf = keys.reshape(-1, D) @ Wk
    xm = (float(np.abs(qf).max()) + float(np.abs(kf).max())) * 1.02
    sig = float(np.sqrt(qf.std() ** 2 + kf.std() ** 2))
    half_period = L_OVER_XM * xm
    coef = _fit_coeffs(xm, M_TERMS, half_period, sig)
    scale = 1.0 / (2.0 * half_period)

    qT = queries.transpose(0, 2, 1).reshape(B, 2, 128, Q).transpose(0, 2, 1, 3)
    kT = keys.transpose(0, 2, 1).reshape(B, 2, 128, K).transpose(0, 2, 1, 3)
    qkT_np = np.ascontiguousarray(np.stack([qT, kT], axis=2).astype(np.float16))
    wq = (Wq * scale).reshape(2, 128, H).transpose(1, 0, 2)
    wk = (Wk * scale).reshape(2, 128, H).transpose(1, 0, 2)
    wqk_np = np.ascontiguousarray(np.stack([wq, wk], axis=1).astype(np.float16))
    ones = np.ones((B, K, 1), np.float32)
    vals_np = np.ascontiguousarray(
        np.concatenate([values, ones], axis=2)
        .reshape(B, 2, 128, D + 1)
        .transpose(0, 2, 1, 3)
        .astype(np.float16)
    )
    beta1 = (float(coef[0]) * wv).reshape(2, 128)
    kidx = np.arange(K)
    maskv = np.where(
        kidx[None, :] < valid_lens[:, None], 0.0, MASK_NEG
    ).astype(np.float32).reshape(B, 2, 128)
    aux_np = np.zeros((N_CORES, 128, 8), np.float32)
    aux_np[:, :, 6] = -2.5
    aux_np[:, :, 7] = -1.5
    for c in range(N_CORES):
        aux_np[c, :, 0] = beta1[0]
        aux_np[c, :, 1] = beta1[1]
        for sl in range(SLOTS):
            for kc in range(2):
                aux_np[c, :, 2 + 2 * sl + kc] = maskv[c * SLOTS + sl, kc]

    return {
        "qkT": qkT_np,
        "wqk": wqk_np,
        "vals": vals_np,
        "aux": aux_np,
        "coef": coef,
    }


def kernel(**inputs) -> np.ndarray:
    global LAST_EXEC_TIME_NS, LAST_RESULTS
    g = _prepare(inputs)
    nc = _get_graph(g["coef"])
    in_maps = []
    for c in range(N_CORES):
        sl = slice(c * SLOTS, (c + 1) * SLOTS)
        in_maps.append(
            {
                "qkT": g["qkT"][sl],
                "wqk": g["wqk"],
                "vals": g["vals"][sl],
                "aux": g["aux"][c],
            }
        )

    res = run_bass_kernel_spmd(nc, in_maps, core_ids=list(range(N_CORES)))
    LAST_EXEC_TIME_NS = res.exec_time_ns
    LAST_RESULTS = res
    out = np.concatenate(
        [np.asarray(res.results[c]["out"]) for c in range(N_CORES)], axis=0
    )
    return out.astype(np.float32)


if __name__ == "__main__":
    import os

    if os.path.exists("/root/problem/inputs_cache.npz"):
        d = np.load("/root/problem/inputs_cache.npz")
        o = kernel(**{k: d[k] for k in d.files})
        exp = np.load("/root/problem/expected_cache.npy")
        rel = np.linalg.norm(o - exp) / np.linalg.norm(exp)
        relmax = np.abs(o - exp).max() / np.abs(exp).max()
        print("rel norm err:", rel, "rel max err:", relmax)
